# revision 34
# baseline (speedup 1.0000x reference)
"""Trainium2 Bass kernel for dual-attention block (CAM + SAM + bottleneck).

Contract: kernel(**inputs) takes FULL unsharded inputs
  x     [8, 64, 64, 64] f32
  w_cam [64, 64], w_q [32, 64], w_k [32, 64], w_v [64, 64], w_bn [64, 128]
and returns the full [8, 64, 64, 64] f32 output.

Sharding: data-parallel over batch across 8 NeuronCores (1 image each);
weights replicated. Per-core math (c=64 channels, n=m=4096 spatial):

  CAM: xcT = x.T @ w_cam.T ; Ec = xcT.T @ xcT;
       attn_c = softmax_rows(Ec); bn = ((wbn1 @ attn_c) + I) @ x
       (the +I folds the residual x into the CAM bottleneck matmul)
  SAM: q4/k4 = (w stacked 4x) @ x  -> q,k replicated on 4 partition groups
       S[m,n] = sum_c k[c,m] q[c,n]  (row-tiled K=32 matmuls on disjoint
       32-row quadrants)
       E = exp(S - ln64) in fp8-e4m3  (max|S|=9.05 -> E'max 133 < 240;
           the 1/64 cancels between numerator and denominator)
       acc[c,n] = sum_m W[m,c] E[m,n]  with W = [v'.T | ones] in fp8 and
                  v' = (wbn2 @ w_v) x  (bottleneck conv folded into the
                  value weights on the host), one DoubleRow matmul per
                  m-tile PAIR (K=256) -> rows 0..63 = final (unnormalized)
                  SAM contribution, row 64 = Z
  out = bn + acc[0:64] * (1/Z)
        (1/Z via the custom-DVE fast approx reciprocal -- partition 0
        only, reached through a 2KB Z-row DMA hop -- then broadcast to 64
        partitions by GpSimd partition_broadcast and applied by one DVE
        multiply)

Design (v16, 221us -> ~165.5us).  ScalarE is the bound: 128 exp
activations of [128,1024] at (1024+352)/1.2GHz = 1147ns each = 142.6us;
everything else is scheduled to keep that stream gapless.  The PE runs
cold (1.2 GHz): the HAM clock gate only lifts to 2.4 GHz after ~3.4us of
gapless matmuls and re-throttles on any exp-paced idle, so a warm-up is
pointless -- the schedule instead keeps cold-PE work per exp window
under the exp duration.  Key structure:
  - spool = ONE 3-slot rotation (3 x 2 PSUM banks) for the S tiles: a
    group's bank frees 3 exp-periods ahead, so all 4 K=32 quadrant
    matmuls of a pair issue back-to-back.  bn / m1ps / q-chunk fills
    ride the same rotation (each extra insertion costs ~1us of exp
    stream -- they are batched and placed where gaps already exist).
  - lag-1 DR emission: a pair's DoubleRow matmuls are emitted after the
    NEXT pair's S matmuls, so at block boundaries the next S quad sits
    in the PE queue ahead of the last DRs and the exp stream never
    head-of-line blocks (this alone was worth 11us).
  - block 0 has no PE budget for its DRs next to the wvc (=[v'|cam] 1x1
    conv) production and k-chunk fills, so its E tiles persist in SBUF
    (e0) and its 16 DRs ride block 1's slack; wvc/fill PSUM scratch uses
    the vacc banks, which are idle in block 0.
  - ec (CAM energy) runs in blocks 2-3 (3 matmuls/pair early) and the
    CAM softmax chain at (3,6) so M1T is ready without stalling block 4.
  - vpool rotation: vacc0(A) vacc1(B) | vacc2(A) EC(B) | vacc3(A) |
    CAM, vacc4(B) | vacc5(A) | vacc6(B) | vacc7(A); each claim lands one
    aux-evacuation behind its slot's previous tenant.
  - epilogues are split recip(+2 blocks) / scale(+3..4) / bn+out(+3..4)
    and spread so only block 7's chain remains after the last exp.
  - preamble: wk4T/wq4T first on the sync/scalar DMA queues (DMA
    completion latency is ~3us regardless of size), x in 8 chunks over 3
    queues, only chunk-0 q/k up front with BOTH evacuations on the
    otherwise-idle ScalarE (DVE is congested with casts); first exp
    ~13.5us.
PSUM: spool 3x2 + vacc/EC 2 = 8 banks.
"""

import sys
from contextlib import ExitStack

import numpy as np

if "/opt/trn_rl_repo" not in sys.path:
    sys.path.insert(0, "/opt/trn_rl_repo")

import concourse.bass as bass
import concourse.tile as tile
from concourse import bacc, mybir
from concourse.bass_utils import run_bass_kernel_spmd

F32 = mybir.dt.float32
BF16 = mybir.dt.bfloat16
FP8 = mybir.dt.float8e4

C = 64          # channels
HW = 4096       # 64*64 spatial
NB = 8          # number of 512-wide n blocks
BLK = 512
MT = 32         # m tiles of 128
NG = 16         # groups of 2 m-tiles per n-block
WP = 80         # wt8 per-m-tile stride (65 used; 80 for DoubleRow step%16==0)
NLOG64 = -4.1588830833596715

Exp = mybir.ActivationFunctionType.Exp
DR = mybir.MatmulPerfMode.DoubleRow


def _build_kernel(ctx: ExitStack, tc: tile.TileContext, io: dict):
    nc = tc.nc
    x_d = io["x"]
    out_d = io["out"]

    consts = ctx.enter_context(tc.tile_pool(name="consts", bufs=1))
    bigs = ctx.enter_context(tc.tile_pool(name="bigs", bufs=1))
    epool = ctx.enter_context(tc.tile_pool(name="epool", bufs=4))
    campool = ctx.enter_context(tc.tile_pool(name="campool", bufs=1))
    sampool = ctx.enter_context(tc.tile_pool(name="sampool", bufs=4))
    spool = ctx.enter_context(
        tc.tile_pool(name="spool", bufs=3, space=bass.MemorySpace.PSUM)
    )
    vpool = ctx.enter_context(
        tc.tile_pool(name="vpool", bufs=2, space=bass.MemorySpace.PSUM)
    )

    # ---- weight DMAs first, on the otherwise-idle GpSimd queue (tiny; if
    # they queued behind the 1MB x transfer the first matmul waits ~15us) --
    wq4T = consts.tile([C, 128], BF16)    # (w_q stacked 4x).T
    wk4T = consts.tile([C, 128], BF16)
    wvc = consts.tile([C, 128], BF16)     # [(wbn2 w_v).T | w_cam.T]
    wbn1T = consts.tile([C, C], F32)
    id64 = consts.tile([C, C], BF16)
    zb = consts.tile([128, 1], F32)
    nlog64 = consts.tile([128, 1], F32)   # exp bias: E'=E/64 fits fp8e4 max 240
    dummy = consts.tile([128, 1], F32)

    nc.sync.dma_start(wk4T[:], io["wk4T"][:])
    nc.scalar.dma_start(wq4T[:], io["wq4T"][:])
    nc.gpsimd.dma_start(wvc[:], io["wvc"][:])
    nc.gpsimd.dma_start(wbn1T[:], io["wbn1T"][:])
    nc.gpsimd.dma_start(id64[:], io["id64"][:])

    # ---- x DMA: 8 column chunks round-robin over 3 HWDGE queues (each
    # queue sustains only ~100 GB/s; chunk 0 -- all that the first S
    # matmuls need -- lands first, right behind wk4T/wq4T) ----
    x_sb = bigs.tile([C, HW], F32)
    x_qs = [nc.sync, nc.scalar, nc.gpsimd]
    for xc_ in range(8):
        x_qs[xc_ % 3].dma_start(
            x_sb[:, xc_ * BLK : (xc_ + 1) * BLK],
            x_d[:, xc_ * BLK : (xc_ + 1) * BLK],
        )

    nc.vector.memset(zb[:], 0.0)
    # Trigger the exp ACT-table load right behind the x-DMA issue (overlaps
    # the transfer) instead of in front of the first real exp.
    nc.scalar.activation(dummy[:], zb[:], Exp, bias=zb[:])
    nc.vector.memset(nlog64[:], NLOG64)

    q4 = bigs.tile([128, HW], BF16)
    k4 = bigs.tile([128, HW], BF16)
    wt8 = bigs.tile([128, MT * WP], FP8)   # per m-tile [v'T | ones | pad]
    xct = bigs.tile([128, MT * C], BF16)   # xcT, m-tile-major
    x_bf = bigs.tile([C, HW], BF16)
    e0 = bigs.tile([128, NG * 2 * BLK], FP8)  # block-0 E, consumed in block 1

    # ones column of wt8 (wvc copies below only write cols 0..63)
    nc.vector.memset(
        wt8[:].rearrange("p (t c) -> p t c", c=WP)[:, :, 64:65], 1.0
    )

    # x in bf16 feeds the q4/k4/wvc/bn matmuls at full PE rate.  Only
    # chunks 0-1 are cast up front: casts for chunks that arrive later are
    # emitted inside the loop so they cannot head-of-line-block the DVE
    # queue in front of the k0/q0 evacuations.
    def x_cast(xc_):
        nc.vector.tensor_copy(
            x_bf[:, xc_ * BLK : (xc_ + 1) * BLK], x_sb[:, xc_ * BLK : (xc_ + 1) * BLK]
        )

    x_cast(0)
    x_cast_sched = {(0, 0): [2], (0, 1): [3], (0, 2): [4], (0, 3): [5],
                    (0, 4): [6], (0, 5): [7]}

    # ---- q4 / k4: replicated q,k via stacked-weight 1x1 convs.  Each
    # 2-chunk group is ONE FD=1024 matmul.  Chunks 0-1 of k and q are
    # produced up front; the rest are fill-in groups inside the block
    # loop, each 2+ pairs ahead of its consumption deadline. ----
    def qk_group(which, cch, nch=1, on_scalar=False, pool=None):
        wT, dst = (wk4T, k4) if which == "k" else (wq4T, q4)
        if pool is None:
            ps = spool.tile([128, nch * BLK], F32, tag="s", name="qkps")
        else:
            ps = pool.tile([128, nch * BLK], F32, tag="v", name="qkps")
        for i in range(nch):
            nc.tensor.matmul(
                ps[:, i * BLK : (i + 1) * BLK],
                wT[:],
                x_bf[:, (cch + i) * BLK : (cch + i + 1) * BLK],
                start=True,
                stop=True,
            )
        lo = cch * BLK
        if on_scalar:
            nc.scalar.copy(dst[:, lo : lo + nch * BLK], ps[:])
        else:
            nc.vector.tensor_copy(dst[:, lo : lo + nch * BLK], ps[:])

    qk_group("k", 0, on_scalar=True)
    qk_group("q", 0, on_scalar=True)
    x_cast(1)

    # (block, pair) -> (which, chunk); deadlines: k chunk c is consumed
    # at block-0 pair c; q chunk c at block c.
    qk_fill = {
        (0, 0): [("k", 1), ("k", 2)], (0, 1): [("k", 3)],
        (0, 2): [("k", 4)], (0, 3): [("k", 5)], (0, 4): [("k", 6)],
        (0, 5): [("k", 7)], (0, 6): [("q", 1)],
        (1, 5): [("q", 2)], (2, 2): [("q", 3, 2)],
        (3, 2): [("q", 5, 2)], (4, 2): [("q", 7)],
    }

    state = {}  # EC tile, allocated at block 1 start (vpool slot timing)

    def wvc_group(base, size):
        """xcT and WT (=[v'T|ones]) production for one m-tile group."""
        ps_w = vpool.tile([128, BLK], F32, tag="v", name="wvcps")
        for j in range(size):
            m = base + j
            nc.tensor.matmul(
                ps_w[:, j * 128 : (j + 1) * 128],
                x_bf[:, m * 128 : (m + 1) * 128],
                wvc[:],
                start=True,
                stop=True,
            )
        src = ps_w[:, : size * 128].rearrange("p (j c) -> p j c", c=128)
        wt_dst = wt8[:, base * WP : (base + size) * WP].rearrange(
            "p (j c) -> p j c", c=WP
        )
        with nc.allow_low_precision(reason="v' in fp8 for DoubleRow acc"):
            nc.vector.tensor_copy(wt_dst[:, :, 0:C], src[:, :, 0:C])
        xct_dst = xct[:, base * C : (base + size) * C].rearrange(
            "p (j c) -> p j c", c=C
        )
        with nc.allow_low_precision(reason="xcT in bf16 for cheap ec matmuls"):
            nc.vector.tensor_copy(xct_dst, src[:, :, C : 2 * C])

    def ec_group(base, size):
        EC = state["EC"]
        for j in range(size):
            m = base + j
            nc.tensor.matmul(
                EC[0:C, 0:C],
                xct[:, m * C : (m + 1) * C],
                xct[:, m * C : (m + 1) * C],
                start=(m == 0),
                stop=(m == MT - 1),
            )

    # ---- per-block state for split epilogues ----
    vaccs = [None] * NB
    sam = [None] * NB   # sam65 [65, BLK] f32: rows 0..63 unnorm SAM out, 64 = Z
    rzs = [None] * NB   # rz [1, BLK] bf16 at partition 0
    M1T_sb = campool.tile([C, C], BF16)

    def epilogue_a(nb):
        """At block end: evacuate vacc (recip is emitted separately)."""
        aux = sampool.tile([C + 1, BLK], F32, tag="aux", name="aux")
        nc.vector.tensor_copy(aux[:], vaccs[nb][0 : C + 1, :])
        sam[nb] = aux

    def emit_recip(nb):
        """1/Z for block nb via the fast approx recip + bf16 cast.

        The custom DVE op only works at base partition 0 (and DVE lanes
        cannot move data across partitions), so the Z row is first moved
        from partition 64 to partition 0 by a tiny SBUF->SBUF DMA on the
        otherwise-idle sync queue.
        """
        z0 = sampool.tile([1, BLK], F32, tag="z0", name="z0")
        nc.sync.dma_start(z0[:], sam[nb][C : C + 1, :])
        rz32 = sampool.tile([1, BLK], F32, tag="rz32", name="rz32")
        nc.vector.reciprocal_approx_fast(rz32[:], z0[:])
        rzb = sampool.tile([1, BLK], BF16, tag="rz", name="rzb")
        with nc.allow_low_precision(reason="1/Z in bf16: 0.4% on the SAM term"):
            nc.vector.tensor_copy(rzb[:], rz32[:])
        rzs[nb] = rzb

    def epilogue_b1(nb):
        """Broadcast 1/Z to 64 partitions (GpSimd) and scale the SAM rows."""
        bcast = sampool.tile([C, BLK], BF16, tag="bc", name="bcast")
        nc.gpsimd.partition_broadcast(bcast[:], rzs[nb][:])
        sam_sc = sampool.tile([C, BLK], F32, tag="sc", name="sam_sc")
        nc.vector.tensor_mul(sam_sc[:], sam[nb][0:C, :], bcast[:])
        return sam_sc

    def epilogue_b2(nb, sam_sc):
        """CAM bottleneck (+residual via I) matmul, add SAM term, DMA out."""
        ncol = slice(nb * BLK, (nb + 1) * BLK)
        bn = spool.tile([128, BLK], F32, tag="s", name="bn")
        nc.tensor.matmul(
            bn[0:C, :], M1T_sb[:], x_bf[:, ncol], start=True, stop=True
        )
        o_t = sampool.tile([C, BLK], F32, tag="ot", name="o_t")
        nc.vector.tensor_add(o_t[:], bn[0:C, :], sam_sc[:])
        nc.sync.dma_start(out_d[:, ncol], o_t[:])

    def cam_chain():
        """CAM softmax -> attn_c -> M1T = (wbn1 @ attn_c).T + I"""
        EC = state["EC"]
        negmax = campool.tile([C, 1], F32)
        nc.vector.reduce_max(
            negmax[:], EC[0:C, 0:C], axis=mybir.AxisListType.X, negate=True
        )
        exp_c = campool.tile([C, C], F32)
        nc.scalar.activation(exp_c[:], EC[0:C, 0:C], Exp, bias=negmax[:])
        sum_c = campool.tile([C, 1], F32)
        nc.vector.reduce_sum(sum_c[:], exp_c[:], axis=mybir.AxisListType.X)
        rec_c = campool.tile([C, 1], F32)
        nc.vector.reciprocal(rec_c[:], sum_c[:])
        attn_c = campool.tile([C, C], F32)
        nc.vector.tensor_scalar_mul(attn_c[:], exp_c[:], rec_c[:])
        m1ps = spool.tile([128, BLK], F32, tag="s", name="m1ps")
        nc.tensor.matmul(
            m1ps[0:C, 0:C], attn_c[:], wbn1T[:], start=True, stop=True
        )
        with nc.allow_low_precision(reason="M1T in bf16 feeds a bf16 matmul"):
            nc.vector.tensor_add(M1T_sb[:], m1ps[0:C, 0:C], id64[:])

    # ---- main SAM loop over 8 n-blocks, groups emitted in PAIRS ----
    # Block 0 carries wvc + k-chunk fill-ins instead of its DR matmuls
    # (which would blow its PE budget); its E tiles persist in e0 and the
    # 16 deferred DRs ride block 1's slack.  ec runs in blocks 2-3 and the
    # CAM chain at block 4; the vacc/EC vpool rotation is:
    #   vacc0(A) vacc1(B) | vacc2(A) EC(B) | vacc3(A) | CAM, vacc4(B) |
    #   vacc5(A) | vacc6(B) | vacc7(A)
    # each claim one aux-evacuation behind its slot's previous tenant.
    sc_pend = {}
    # ec m-tile coverage: 3/pair on block-2 pairs 0-3, then 2/pair; all 32
    # done by block-3 pair 5 so the CAM chain can run at (3,6).
    ec_sched = {(2, 0): (0, 3), (2, 1): (3, 3), (2, 2): (6, 3),
                (2, 3): (9, 3), (2, 4): (12, 2), (2, 5): (14, 2),
                (2, 6): (16, 2), (2, 7): (18, 2), (3, 0): (20, 2),
                (3, 1): (22, 2), (3, 2): (24, 2), (3, 3): (26, 2),
                (3, 4): (28, 2), (3, 5): (30, 2)}
    recip_sched = {(2, 0): 0, (2, 4): 1, (3, 0): 2, (4, 0): 3,
                   (5, 0): 4, (6, 0): 5, (7, 0): 6}
    b1_sched = {(4, 4): 0, (5, 1): 1, (5, 5): 2, (6, 1): 3,
                (6, 5): 4, (7, 1): 5, (7, 4): 6}
    b2_sched = {(4, 6): 0, (5, 3): 1, (5, 7): 2, (6, 3): 3,
                (6, 7): 4, (7, 3): 5, (7, 6): 6}
    pend_dr = None  # (nb, [(g, e_t)...], is_last_pair)

    def emit_drs(dnb, ges, _last):
        for g, e_t in ges:
            lhsT = wt8[:, 2 * g * WP : (2 * g + 2) * WP].rearrange(
                "p (two f) -> p two f", two=2
            )[:, :, 0:65]
            rhs = e_t[:].rearrange("p (two f) -> p two f", two=2)
            nc.tensor.matmul(
                vaccs[dnb][0 : C + 1, :],
                lhsT,
                rhs,
                start=(g == 0),
                stop=(g == NG - 1),
                perf_mode=DR,
            )

    for nb in range(NB):
        if nb == 1:
            vaccs[0] = vpool.tile([128, BLK], F32, tag="v", name="vacc0")
        if nb != 0:
            vacc = vpool.tile([128, BLK], F32, tag="v", name="vacc")
            vaccs[nb] = vacc
        if nb == 2:
            # EC right after vacc2: slot B, re-claimed by vacc4 after the
            # CAM chain at block 4 has consumed EC.
            state["EC"] = vpool.tile([128, BLK], F32, tag="v", name="EC")
        ncol = slice(nb * BLK, (nb + 1) * BLK)
        for p in range(NG // 2):
            g0, g1 = 2 * p, 2 * p + 1
            s_ts = []
            for g in (g0, g1):
                s_t = spool.tile([128, 2 * BLK], F32, tag="s", name="s_t")
                s_ts.append(s_t)
                for j in range(2):
                    m = 2 * g + j
                    r = 2 * (g % 2) + j  # row quadrants 0,1 / 2,3
                    nc.tensor.matmul(
                        s_t[:, j * BLK : (j + 1) * BLK],
                        k4[32 * r : 32 * r + 32, m * 128 : (m + 1) * 128],
                        q4[32 * r : 32 * r + 32, ncol],
                        start=True,
                        stop=True,
                        tile_position=(32 * r, 0),
                    )
            if nb == 0:
                wvc_group(2 * g0, 2)
                wvc_group(2 * g1, 2)
            if (nb, p) in ec_sched:
                ec_group(*ec_sched[(nb, p)])
            if (nb, p) == (3, 6):
                cam_chain()
            for xc2 in x_cast_sched.get((nb, p), ()):
                x_cast(xc2)
            for fill in qk_fill.get((nb, p), ()):
                qk_group(*fill, pool=vpool if nb == 0 else None)
            # lag-1 DR emission: the previous pair's DR matmuls are emitted
            # AFTER this pair's S matmuls, so at block boundaries the next
            # block's S quad is in the PE queue ahead of the last DRs and
            # the exp stream never head-of-line blocks on them.
            if pend_dr is not None:
                emit_drs(*pend_dr)
                if pend_dr[2]:
                    epilogue_a(pend_dr[0])
                pend_dr = None
            e_ts = []
            for g, s_t in zip((g0, g1), s_ts):
                if nb == 0:
                    e_t = e0[:, 2 * g * BLK : 2 * (g + 1) * BLK]
                else:
                    e_t = epool.tile([128, 2 * BLK], FP8, tag="e", name="e_t")
                e_ts.append(e_t)
                with nc.allow_low_precision(reason="E in fp8: ~1e-4 on out"):
                    nc.scalar.activation(e_t[:], s_t[:], Exp, bias=nlog64[:])
            if nb != 0:
                pend_dr = (nb, list(zip((g0, g1), e_ts)), p == NG // 2 - 1)
            if nb == 1:
                # block 0's deferred DR matmuls, 2 per pair
                for g in (g0, g1):
                    lhsT = wt8[:, 2 * g * WP : (2 * g + 2) * WP].rearrange(
                        "p (two f) -> p two f", two=2
                    )[:, :, 0:65]
                    rhs = e0[:, 2 * g * BLK : 2 * (g + 1) * BLK].rearrange(
                        "p (two f) -> p two f", two=2
                    )
                    nc.tensor.matmul(
                        vaccs[0][0 : C + 1, :],
                        lhsT,
                        rhs,
                        start=(g == 0),
                        stop=(g == NG - 1),
                        perf_mode=DR,
                    )
            if (nb, p) in recip_sched:
                emit_recip(recip_sched[(nb, p)])
            if (nb, p) in b1_sched:
                s = b1_sched[(nb, p)]
                sc_pend[s] = epilogue_b1(s)
            if (nb, p) in b2_sched:
                s = b2_sched[(nb, p)]
                epilogue_b2(s, sc_pend.pop(s))

        if nb == 1:
            epilogue_a(0)
    if pend_dr is not None:
        emit_drs(*pend_dr)
        if pend_dr[2]:
            epilogue_a(pend_dr[0])
        pend_dr = None

    # ---- tail: only block 7's chain remains.  The Z-row DMA reads the
    # PSUM accumulator directly (safe here: vacc7 is never recycled), so
    # the reciprocal chain starts without waiting for the aux copy. ----
    emit_recip(NB - 1)
    # block 7's bcast/scale/add/DMA run in column halves so the GpSimd
    # broadcast, DVE ops, and out-DMA pipeline instead of paying
    # full-width latency serially (the Z-row DMA + recip stay full-width
    # -- their ~1.7us DMA latency does not subdivide).
    nb = NB - 1
    ncol0 = nb * BLK
    H = BLK // 2
    bn7 = spool.tile([128, BLK], F32, tag="s", name="bn7")
    nc.tensor.matmul(
        bn7[0:C, :], M1T_sb[:], x_bf[:, ncol0 : ncol0 + BLK],
        start=True, stop=True,
    )
    for h in range(2):
        cs = slice(h * H, (h + 1) * H)
        bc_h = sampool.tile([C, H], BF16, tag=f"bct{h}", name="bct")
        nc.gpsimd.partition_broadcast(bc_h[:], rzs[nb][0:1, cs])
        sc_h = sampool.tile([C, H], F32, tag=f"sct{h}", name="sct")
        nc.vector.tensor_mul(sc_h[:], sam[nb][0:C, cs], bc_h[:])
        o_h = sampool.tile([C, H], F32, tag=f"ott{h}", name="ott")
        nc.vector.tensor_add(o_h[:], bn7[0:C, cs], sc_h[:])
        nc.sync.dma_start(out_d[:, ncol0 + h * H : ncol0 + (h + 1) * H], o_h[:])


def build_nc():
    nc = bacc.Bacc(
        "TRN2",
        target_bir_lowering=False,
        debug=False,
        enable_asserts=False,
        num_devices=8,
    )
    io = {}
    io["x"] = nc.dram_tensor("x", [C, HW], F32, kind="ExternalInput").ap()
    io["wq4T"] = nc.dram_tensor("wq4T", [C, 128], BF16, kind="ExternalInput").ap()
    io["wk4T"] = nc.dram_tensor("wk4T", [C, 128], BF16, kind="ExternalInput").ap()
    io["wvc"] = nc.dram_tensor("wvc", [C, 128], BF16, kind="ExternalInput").ap()
    io["wbn1T"] = nc.dram_tensor("wbn1T", [C, C], F32, kind="ExternalInput").ap()
    io["id64"] = nc.dram_tensor("id64", [C, C], BF16, kind="ExternalInput").ap()
    io["out"] = nc.dram_tensor("out", [C, HW], F32, kind="ExternalOutput").ap()

    with tile.TileContext(nc) as tc:
        with ExitStack() as ctx:
            _build_kernel(ctx, tc, io)
    nc.compile()
    return nc


def make_in_maps(x, w_cam, w_q, w_k, w_v, w_bn):
    import ml_dtypes

    f = lambda a: np.ascontiguousarray(np.asarray(a, dtype=np.float32))
    fb = lambda a: np.ascontiguousarray(
        np.asarray(a, dtype=np.float32).astype(ml_dtypes.bfloat16)
    )
    w_bn = np.asarray(w_bn, dtype=np.float64)
    w_vp = w_bn[:, C:] @ np.asarray(w_v, dtype=np.float64)  # wbn2 folded into v
    base = {
        "wq4T": fb(np.concatenate([np.asarray(w_q).T] * 4, axis=1)),
        "wk4T": fb(np.concatenate([np.asarray(w_k).T] * 4, axis=1)),
        "wvc": fb(np.concatenate([w_vp.T, np.asarray(w_cam).T], axis=1)),
        "wbn1T": f(w_bn[:, :C].T),
        "id64": fb(np.eye(C)),
    }
    x = np.asarray(x)
    return [dict(base, x=f(x[b].reshape(C, HW))) for b in range(8)]


_NC_CACHE = None


def kernel(x, w_cam, w_q, w_k, w_v, w_bn):
    global _NC_CACHE
    if _NC_CACHE is None:
        _NC_CACHE = build_nc()
    nc = _NC_CACHE
    in_maps = make_in_maps(x, w_cam, w_q, w_k, w_v, w_bn)
    res = run_bass_kernel_spmd(nc, in_maps, list(range(8)))
    out = np.stack([res.results[b]["out"].reshape(C, 64, 64) for b in range(8)])
    return out.astype(np.float32)


# revision 35
# speedup vs baseline: 1.0005x; 1.0005x over previous
"""Trainium2 Bass kernel for dual-attention block (CAM + SAM + bottleneck).

Contract: kernel(**inputs) takes FULL unsharded inputs
  x     [8, 64, 64, 64] f32
  w_cam [64, 64], w_q [32, 64], w_k [32, 64], w_v [64, 64], w_bn [64, 128]
and returns the full [8, 64, 64, 64] f32 output.

Sharding: data-parallel over batch across 8 NeuronCores (1 image each);
weights replicated. Per-core math (c=64 channels, n=m=4096 spatial):

  CAM: xcT = x.T @ w_cam.T ; Ec = xcT.T @ xcT;
       attn_c = softmax_rows(Ec); bn = ((wbn1 @ attn_c) + I) @ x
       (the +I folds the residual x into the CAM bottleneck matmul)
  SAM: q4/k4 = (w stacked 4x) @ x  -> q,k replicated on 4 partition groups
       S[m,n] = sum_c k[c,m] q[c,n]  (row-tiled K=32 matmuls on disjoint
       32-row quadrants)
       E = exp(S - ln64) in fp8-e4m3  (max|S|=9.05 -> E'max 133 < 240;
           the 1/64 cancels between numerator and denominator)
       acc[c,n] = sum_m W[m,c] E[m,n]  with W = [v'.T | ones] in fp8 and
                  v' = (wbn2 @ w_v) x  (bottleneck conv folded into the
                  value weights on the host), one DoubleRow matmul per
                  m-tile PAIR (K=256) -> rows 0..63 = final (unnormalized)
                  SAM contribution, row 64 = Z
  out = bn + acc[0:64] * (1/Z)
        (1/Z via the custom-DVE fast approx reciprocal -- partition 0
        only, reached through a 2KB Z-row DMA hop -- then broadcast to 64
        partitions by GpSimd partition_broadcast and applied by one DVE
        multiply)

Design (v16, 221us -> ~165.5us).  ScalarE is the bound: 128 exp
activations of [128,1024] at (1024+352)/1.2GHz = 1147ns each = 142.6us;
everything else is scheduled to keep that stream gapless.  The PE runs
cold (1.2 GHz): the HAM clock gate only lifts to 2.4 GHz after ~3.4us of
gapless matmuls and re-throttles on any exp-paced idle, so a warm-up is
pointless -- the schedule instead keeps cold-PE work per exp window
under the exp duration.  Key structure:
  - spool = ONE 3-slot rotation (3 x 2 PSUM banks) for the S tiles: a
    group's bank frees 3 exp-periods ahead, so all 4 K=32 quadrant
    matmuls of a pair issue back-to-back.  bn / m1ps / q-chunk fills
    ride the same rotation (each extra insertion costs ~1us of exp
    stream -- they are batched and placed where gaps already exist).
  - lag-1 DR emission: a pair's DoubleRow matmuls are emitted after the
    NEXT pair's S matmuls, so at block boundaries the next S quad sits
    in the PE queue ahead of the last DRs and the exp stream never
    head-of-line blocks (this alone was worth 11us).
  - block 0 has no PE budget for its DRs next to the wvc (=[v'|cam] 1x1
    conv) production and k-chunk fills, so its E tiles persist in SBUF
    (e0) and its 16 DRs ride block 1's slack; wvc/fill PSUM scratch uses
    the vacc banks, which are idle in block 0.
  - ec (CAM energy) runs in blocks 2-3 (3 matmuls/pair early) and the
    CAM softmax chain at (3,6) so M1T is ready without stalling block 4.
  - vpool rotation: vacc0(A) vacc1(B) | vacc2(A) EC(B) | vacc3(A) |
    CAM, vacc4(B) | vacc5(A) | vacc6(B) | vacc7(A); each claim lands one
    aux-evacuation behind its slot's previous tenant.
  - epilogues are split recip(+2 blocks) / scale(+3..4) / bn+out(+3..4)
    and spread so only block 7's chain remains after the last exp.
  - preamble: wk4T/wq4T first on the sync/scalar DMA queues (DMA
    completion latency is ~3us regardless of size), x in 8 chunks over 3
    queues, only chunk-0 q/k up front with BOTH evacuations on the
    otherwise-idle ScalarE (DVE is congested with casts); first exp
    ~13.5us.
PSUM: spool 3x2 + vacc/EC 2 = 8 banks.
"""

import sys
from contextlib import ExitStack

import numpy as np

if "/opt/trn_rl_repo" not in sys.path:
    sys.path.insert(0, "/opt/trn_rl_repo")

import concourse.bass as bass
import concourse.tile as tile
from concourse import bacc, mybir
from concourse.bass_utils import run_bass_kernel_spmd

F32 = mybir.dt.float32
BF16 = mybir.dt.bfloat16
FP8 = mybir.dt.float8e4

C = 64          # channels
HW = 4096       # 64*64 spatial
NB = 8          # number of 512-wide n blocks
BLK = 512
MT = 32         # m tiles of 128
NG = 16         # groups of 2 m-tiles per n-block
WP = 80         # wt8 per-m-tile stride (65 used; 80 for DoubleRow step%16==0)
NLOG64 = -4.1588830833596715

Exp = mybir.ActivationFunctionType.Exp
DR = mybir.MatmulPerfMode.DoubleRow


def _build_kernel(ctx: ExitStack, tc: tile.TileContext, io: dict):
    nc = tc.nc
    x_d = io["x"]
    out_d = io["out"]

    consts = ctx.enter_context(tc.tile_pool(name="consts", bufs=1))
    bigs = ctx.enter_context(tc.tile_pool(name="bigs", bufs=1))
    epool = ctx.enter_context(tc.tile_pool(name="epool", bufs=4))
    campool = ctx.enter_context(tc.tile_pool(name="campool", bufs=1))
    sampool = ctx.enter_context(tc.tile_pool(name="sampool", bufs=4))
    spool = ctx.enter_context(
        tc.tile_pool(name="spool", bufs=3, space=bass.MemorySpace.PSUM)
    )
    vpool = ctx.enter_context(
        tc.tile_pool(name="vpool", bufs=2, space=bass.MemorySpace.PSUM)
    )

    # ---- weight DMAs first, on the otherwise-idle GpSimd queue (tiny; if
    # they queued behind the 1MB x transfer the first matmul waits ~15us) --
    wq4T = consts.tile([C, 128], BF16)    # (w_q stacked 4x).T
    wk4T = consts.tile([C, 128], BF16)
    wvc = consts.tile([C, 128], BF16)     # [(wbn2 w_v).T | w_cam.T]
    wbn1T = consts.tile([C, C], F32)
    id64 = consts.tile([C, C], BF16)
    zb = consts.tile([128, 1], F32)
    nlog64 = consts.tile([128, 1], F32)   # exp bias: E'=E/64 fits fp8e4 max 240
    dummy = consts.tile([128, 1], F32)

    nc.sync.dma_start(wk4T[:], io["wk4T"][:])
    nc.scalar.dma_start(wq4T[:], io["wq4T"][:])
    nc.gpsimd.dma_start(wvc[:], io["wvc"][:])
    nc.gpsimd.dma_start(wbn1T[:], io["wbn1T"][:])
    nc.gpsimd.dma_start(id64[:], io["id64"][:])

    # ---- x DMA: 8 column chunks round-robin over 3 HWDGE queues (each
    # queue sustains only ~100 GB/s; chunk 0 -- all that the first S
    # matmuls need -- lands first, right behind wk4T/wq4T) ----
    x_sb = bigs.tile([C, HW], F32)
    x_qs = [nc.sync, nc.scalar, nc.gpsimd]
    for xc_ in range(8):
        x_qs[xc_ % 3].dma_start(
            x_sb[:, xc_ * BLK : (xc_ + 1) * BLK],
            x_d[:, xc_ * BLK : (xc_ + 1) * BLK],
        )

    nc.vector.memset(zb[:], 0.0)
    # Trigger the exp ACT-table load right behind the x-DMA issue (overlaps
    # the transfer) instead of in front of the first real exp.
    nc.scalar.activation(dummy[:], zb[:], Exp, bias=zb[:])
    nc.vector.memset(nlog64[:], NLOG64)

    q4 = bigs.tile([128, HW], BF16)
    k4 = bigs.tile([128, HW], BF16)
    wt8 = bigs.tile([128, MT * WP], FP8)   # per m-tile [v'T | ones | pad]
    xct = bigs.tile([128, MT * C], BF16)   # xcT, m-tile-major
    x_bf = bigs.tile([C, HW], BF16)
    e0 = bigs.tile([128, NG * 2 * BLK], FP8)  # block-0 E, consumed in block 1

    # ones column of wt8 (wvc copies below only write cols 0..63)
    nc.vector.memset(
        wt8[:].rearrange("p (t c) -> p t c", c=WP)[:, :, 64:65], 1.0
    )

    # x in bf16 feeds the q4/k4/wvc/bn matmuls at full PE rate.  Only
    # chunks 0-1 are cast up front: casts for chunks that arrive later are
    # emitted inside the loop so they cannot head-of-line-block the DVE
    # queue in front of the k0/q0 evacuations.
    def x_cast(xc_):
        nc.vector.tensor_copy(
            x_bf[:, xc_ * BLK : (xc_ + 1) * BLK], x_sb[:, xc_ * BLK : (xc_ + 1) * BLK]
        )

    x_cast(0)
    x_cast_sched = {(0, 0): [2], (0, 1): [3], (0, 2): [4], (0, 3): [5],
                    (0, 4): [6], (0, 5): [7]}

    # ---- q4 / k4: replicated q,k via stacked-weight 1x1 convs.  Each
    # 2-chunk group is ONE FD=1024 matmul.  Chunks 0-1 of k and q are
    # produced up front; the rest are fill-in groups inside the block
    # loop, each 2+ pairs ahead of its consumption deadline. ----
    def qk_group(which, cch, nch=1, on_scalar=False, pool=None):
        wT, dst = (wk4T, k4) if which == "k" else (wq4T, q4)
        if pool is None:
            ps = spool.tile([128, nch * BLK], F32, tag="s", name="qkps")
        else:
            ps = pool.tile([128, nch * BLK], F32, tag="v", name="qkps")
        for i in range(nch):
            nc.tensor.matmul(
                ps[:, i * BLK : (i + 1) * BLK],
                wT[:],
                x_bf[:, (cch + i) * BLK : (cch + i + 1) * BLK],
                start=True,
                stop=True,
            )
        lo = cch * BLK
        if on_scalar:
            nc.scalar.copy(dst[:, lo : lo + nch * BLK], ps[:])
        else:
            nc.vector.tensor_copy(dst[:, lo : lo + nch * BLK], ps[:])

    qk_group("k", 0, on_scalar=True)
    qk_group("q", 0, on_scalar=True)
    x_cast(1)

    # (block, pair) -> (which, chunk); deadlines: k chunk c is consumed
    # at block-0 pair c; q chunk c at block c.
    qk_fill = {
        (0, 0): [("k", 1), ("k", 2)], (0, 1): [("k", 3)],
        (0, 2): [("k", 4)], (0, 3): [("k", 5)], (0, 4): [("k", 6)],
        (0, 5): [("k", 7)], (0, 6): [("q", 1)],
        (1, 5): [("q", 2)], (2, 2): [("q", 3, 2)],
        (3, 2): [("q", 5, 2)], (4, 2): [("q", 7)],
    }

    state = {}  # EC tile, allocated at block 1 start (vpool slot timing)

    def wvc_group(base, size):
        """xcT and WT (=[v'T|ones]) production for one m-tile group."""
        ps_w = vpool.tile([128, BLK], F32, tag="v", name="wvcps")
        for j in range(size):
            m = base + j
            nc.tensor.matmul(
                ps_w[:, j * 128 : (j + 1) * 128],
                x_bf[:, m * 128 : (m + 1) * 128],
                wvc[:],
                start=True,
                stop=True,
            )
        src = ps_w[:, : size * 128].rearrange("p (j c) -> p j c", c=128)
        wt_dst = wt8[:, base * WP : (base + size) * WP].rearrange(
            "p (j c) -> p j c", c=WP
        )
        with nc.allow_low_precision(reason="v' in fp8 for DoubleRow acc"):
            nc.vector.tensor_copy(wt_dst[:, :, 0:C], src[:, :, 0:C])
        xct_dst = xct[:, base * C : (base + size) * C].rearrange(
            "p (j c) -> p j c", c=C
        )
        with nc.allow_low_precision(reason="xcT in bf16 for cheap ec matmuls"):
            nc.vector.tensor_copy(xct_dst, src[:, :, C : 2 * C])

    def ec_group(base, size):
        EC = state["EC"]
        for j in range(size):
            m = base + j
            nc.tensor.matmul(
                EC[0:C, 0:C],
                xct[:, m * C : (m + 1) * C],
                xct[:, m * C : (m + 1) * C],
                start=(m == 0),
                stop=(m == MT - 1),
            )

    # ---- per-block state for split epilogues ----
    vaccs = [None] * NB
    sam = [None] * NB   # sam65 [65, BLK] f32: rows 0..63 unnorm SAM out, 64 = Z
    rzs = [None] * NB   # rz [1, BLK] bf16 at partition 0
    M1T_sb = campool.tile([C, C], BF16)

    def epilogue_a(nb):
        """At block end: evacuate vacc (recip is emitted separately)."""
        aux = sampool.tile([C + 1, BLK], F32, tag="aux", name="aux")
        nc.vector.tensor_copy(aux[:], vaccs[nb][0 : C + 1, :])
        sam[nb] = aux

    def emit_recip(nb):
        """1/Z for block nb via the fast approx recip + bf16 cast.

        The custom DVE op only works at base partition 0 (and DVE lanes
        cannot move data across partitions), so the Z row is first moved
        from partition 64 to partition 0 by a tiny SBUF->SBUF DMA on the
        otherwise-idle sync queue.
        """
        z0 = sampool.tile([1, BLK], F32, tag="z0", name="z0")
        nc.sync.dma_start(z0[:], sam[nb][C : C + 1, :])
        rz32 = sampool.tile([1, BLK], F32, tag="rz32", name="rz32")
        nc.vector.reciprocal_approx_fast(rz32[:], z0[:])
        rzb = sampool.tile([1, BLK], BF16, tag="rz", name="rzb")
        with nc.allow_low_precision(reason="1/Z in bf16: 0.4% on the SAM term"):
            nc.vector.tensor_copy(rzb[:], rz32[:])
        rzs[nb] = rzb

    def epilogue_b1(nb):
        """Broadcast 1/Z to 64 partitions (GpSimd) and scale the SAM rows."""
        bcast = sampool.tile([C, BLK], BF16, tag="bc", name="bcast")
        nc.gpsimd.partition_broadcast(bcast[:], rzs[nb][:])
        sam_sc = sampool.tile([C, BLK], F32, tag="sc", name="sam_sc")
        nc.vector.tensor_mul(sam_sc[:], sam[nb][0:C, :], bcast[:])
        return sam_sc

    def epilogue_b2(nb, sam_sc):
        """CAM bottleneck (+residual via I) matmul, add SAM term, DMA out."""
        ncol = slice(nb * BLK, (nb + 1) * BLK)
        bn = spool.tile([128, BLK], F32, tag="s", name="bn")
        nc.tensor.matmul(
            bn[0:C, :], M1T_sb[:], x_bf[:, ncol], start=True, stop=True
        )
        o_t = sampool.tile([C, BLK], F32, tag="ot", name="o_t")
        nc.vector.tensor_add(o_t[:], bn[0:C, :], sam_sc[:])
        nc.sync.dma_start(out_d[:, ncol], o_t[:])

    def cam_chain():
        """CAM softmax -> attn_c -> M1T = (wbn1 @ attn_c).T + I"""
        EC = state["EC"]
        negmax = campool.tile([C, 1], F32)
        nc.vector.reduce_max(
            negmax[:], EC[0:C, 0:C], axis=mybir.AxisListType.X, negate=True
        )
        exp_c = campool.tile([C, C], F32)
        nc.scalar.activation(exp_c[:], EC[0:C, 0:C], Exp, bias=negmax[:])
        sum_c = campool.tile([C, 1], F32)
        nc.vector.reduce_sum(sum_c[:], exp_c[:], axis=mybir.AxisListType.X)
        rec_c = campool.tile([C, 1], F32)
        nc.vector.reciprocal(rec_c[:], sum_c[:])
        attn_c = campool.tile([C, C], F32)
        nc.vector.tensor_scalar_mul(attn_c[:], exp_c[:], rec_c[:])
        m1ps = spool.tile([128, BLK], F32, tag="s", name="m1ps")
        nc.tensor.matmul(
            m1ps[0:C, 0:C], attn_c[:], wbn1T[:], start=True, stop=True
        )
        with nc.allow_low_precision(reason="M1T in bf16 feeds a bf16 matmul"):
            nc.vector.tensor_add(M1T_sb[:], m1ps[0:C, 0:C], id64[:])

    # ---- main SAM loop over 8 n-blocks, groups emitted in PAIRS ----
    # Block 0 carries wvc + k-chunk fill-ins instead of its DR matmuls
    # (which would blow its PE budget); its E tiles persist in e0 and the
    # 16 deferred DRs ride block 1's slack.  ec runs in blocks 2-3 and the
    # CAM chain at block 4; the vacc/EC vpool rotation is:
    #   vacc0(A) vacc1(B) | vacc2(A) EC(B) | vacc3(A) | CAM, vacc4(B) |
    #   vacc5(A) | vacc6(B) | vacc7(A)
    # each claim one aux-evacuation behind its slot's previous tenant.
    sc_pend = {}
    # ec m-tile coverage: 3/pair on block-2 pairs 0-3, then 2/pair; all 32
    # done by block-3 pair 5 so the CAM chain can run at (3,6).
    ec_sched = {(2, 0): (0, 3), (2, 1): (3, 3), (2, 2): (6, 3),
                (2, 3): (9, 3), (2, 4): (12, 2), (2, 5): (14, 2),
                (2, 6): (16, 2), (2, 7): (18, 2), (3, 0): (20, 2),
                (3, 1): (22, 2), (3, 2): (24, 2), (3, 3): (26, 2),
                (3, 4): (28, 2), (3, 5): (30, 2)}
    recip_sched = {(2, 0): 0, (2, 4): 1, (3, 0): 2, (4, 0): 3,
                   (5, 0): 4, (6, 0): 5, (7, 0): 6}
    b1_sched = {(4, 4): 0, (5, 1): 1, (5, 5): 2, (6, 1): 3,
                (6, 5): 4, (7, 1): 5, (7, 4): 6}
    b2_sched = {(4, 6): 0, (5, 3): 1, (5, 7): 2, (6, 3): 3,
                (6, 7): 4, (7, 3): 5, (7, 6): 6}
    pend_dr = None  # (nb, [(g, e_t)...], is_last_pair)

    def emit_drs(dnb, ges, _last):
        for g, e_t in ges:
            lhsT = wt8[:, 2 * g * WP : (2 * g + 2) * WP].rearrange(
                "p (two f) -> p two f", two=2
            )[:, :, 0:65]
            rhs = e_t[:].rearrange("p (two f) -> p two f", two=2)
            nc.tensor.matmul(
                vaccs[dnb][0 : C + 1, :],
                lhsT,
                rhs,
                start=(g == 0),
                stop=(g == NG - 1),
                perf_mode=DR,
            )

    for nb in range(NB):
        if nb == 1:
            vaccs[0] = vpool.tile([128, BLK], F32, tag="v", name="vacc0")
        if nb != 0:
            vacc = vpool.tile([128, BLK], F32, tag="v", name="vacc")
            vaccs[nb] = vacc
        if nb == 2:
            # EC right after vacc2: slot B, re-claimed by vacc4 after the
            # CAM chain at block 4 has consumed EC.
            state["EC"] = vpool.tile([128, BLK], F32, tag="v", name="EC")
        ncol = slice(nb * BLK, (nb + 1) * BLK)
        for p in range(NG // 2):
            g0, g1 = 2 * p, 2 * p + 1
            s_ts = []
            for g in (g0, g1):
                s_t = spool.tile([128, 2 * BLK], F32, tag="s", name="s_t")
                s_ts.append(s_t)
                for j in range(2):
                    m = 2 * g + j
                    r = 2 * (g % 2) + j  # row quadrants 0,1 / 2,3
                    nc.tensor.matmul(
                        s_t[:, j * BLK : (j + 1) * BLK],
                        k4[32 * r : 32 * r + 32, m * 128 : (m + 1) * 128],
                        q4[32 * r : 32 * r + 32, ncol],
                        start=True,
                        stop=True,
                        tile_position=(32 * r, 0),
                    )
            if nb == 0:
                wvc_group(2 * g0, 2)
                wvc_group(2 * g1, 2)
            if (nb, p) in ec_sched:
                ec_group(*ec_sched[(nb, p)])
            if (nb, p) == (3, 6):
                cam_chain()
            for xc2 in x_cast_sched.get((nb, p), ()):
                x_cast(xc2)
            for fill in qk_fill.get((nb, p), ()):
                qk_group(*fill, pool=vpool if nb == 0 else None)
            # lag-1 DR emission: the previous pair's DR matmuls are emitted
            # AFTER this pair's S matmuls, so at block boundaries the next
            # block's S quad is in the PE queue ahead of the last DRs and
            # the exp stream never head-of-line blocks on them.
            if pend_dr is not None:
                emit_drs(*pend_dr)
                if pend_dr[2]:
                    epilogue_a(pend_dr[0])
                pend_dr = None
            e_ts = []
            for g, s_t in zip((g0, g1), s_ts):
                if nb == 0:
                    e_t = e0[:, 2 * g * BLK : 2 * (g + 1) * BLK]
                else:
                    e_t = epool.tile([128, 2 * BLK], FP8, tag="e", name="e_t")
                e_ts.append(e_t)
                with nc.allow_low_precision(reason="E in fp8: ~1e-4 on out"):
                    nc.scalar.activation(e_t[:], s_t[:], Exp, bias=nlog64[:])
            if nb != 0:
                pend_dr = (nb, list(zip((g0, g1), e_ts)), p == NG // 2 - 1)
            if nb == 1:
                # block 0's deferred DR matmuls, 2 per pair
                for g in (g0, g1):
                    lhsT = wt8[:, 2 * g * WP : (2 * g + 2) * WP].rearrange(
                        "p (two f) -> p two f", two=2
                    )[:, :, 0:65]
                    rhs = e0[:, 2 * g * BLK : 2 * (g + 1) * BLK].rearrange(
                        "p (two f) -> p two f", two=2
                    )
                    nc.tensor.matmul(
                        vaccs[0][0 : C + 1, :],
                        lhsT,
                        rhs,
                        start=(g == 0),
                        stop=(g == NG - 1),
                        perf_mode=DR,
                    )
            if (nb, p) in recip_sched:
                emit_recip(recip_sched[(nb, p)])
            if (nb, p) in b1_sched:
                s = b1_sched[(nb, p)]
                sc_pend[s] = epilogue_b1(s)
            if (nb, p) in b2_sched:
                s = b2_sched[(nb, p)]
                epilogue_b2(s, sc_pend.pop(s))

        if nb == 1:
            epilogue_a(0)
    if pend_dr is not None:
        emit_drs(*pend_dr)
        if pend_dr[2]:
            epilogue_a(pend_dr[0])
        pend_dr = None

    # ---- tail: only block 7's chain remains.  The Z-row DMA reads the
    # PSUM accumulator directly (safe here: vacc7 is never recycled), so
    # the reciprocal chain starts without waiting for the aux copy. ----
    emit_recip(NB - 1)
    sc7 = epilogue_b1(NB - 1)
    epilogue_b2(NB - 1, sc7)


def build_nc():
    nc = bacc.Bacc(
        "TRN2",
        target_bir_lowering=False,
        debug=False,
        enable_asserts=False,
        num_devices=8,
    )
    io = {}
    io["x"] = nc.dram_tensor("x", [C, HW], F32, kind="ExternalInput").ap()
    io["wq4T"] = nc.dram_tensor("wq4T", [C, 128], BF16, kind="ExternalInput").ap()
    io["wk4T"] = nc.dram_tensor("wk4T", [C, 128], BF16, kind="ExternalInput").ap()
    io["wvc"] = nc.dram_tensor("wvc", [C, 128], BF16, kind="ExternalInput").ap()
    io["wbn1T"] = nc.dram_tensor("wbn1T", [C, C], F32, kind="ExternalInput").ap()
    io["id64"] = nc.dram_tensor("id64", [C, C], BF16, kind="ExternalInput").ap()
    io["out"] = nc.dram_tensor("out", [C, HW], F32, kind="ExternalOutput").ap()

    with tile.TileContext(nc) as tc:
        with ExitStack() as ctx:
            _build_kernel(ctx, tc, io)
    nc.compile()
    return nc


def make_in_maps(x, w_cam, w_q, w_k, w_v, w_bn):
    import ml_dtypes

    f = lambda a: np.ascontiguousarray(np.asarray(a, dtype=np.float32))
    fb = lambda a: np.ascontiguousarray(
        np.asarray(a, dtype=np.float32).astype(ml_dtypes.bfloat16)
    )
    w_bn = np.asarray(w_bn, dtype=np.float64)
    w_vp = w_bn[:, C:] @ np.asarray(w_v, dtype=np.float64)  # wbn2 folded into v
    base = {
        "wq4T": fb(np.concatenate([np.asarray(w_q).T] * 4, axis=1)),
        "wk4T": fb(np.concatenate([np.asarray(w_k).T] * 4, axis=1)),
        "wvc": fb(np.concatenate([w_vp.T, np.asarray(w_cam).T], axis=1)),
        "wbn1T": f(w_bn[:, :C].T),
        "id64": fb(np.eye(C)),
    }
    x = np.asarray(x)
    return [dict(base, x=f(x[b].reshape(C, HW))) for b in range(8)]


_NC_CACHE = None


def kernel(x, w_cam, w_q, w_k, w_v, w_bn):
    global _NC_CACHE
    if _NC_CACHE is None:
        _NC_CACHE = build_nc()
    nc = _NC_CACHE
    in_maps = make_in_maps(x, w_cam, w_q, w_k, w_v, w_bn)
    res = run_bass_kernel_spmd(nc, in_maps, list(range(8)))
    out = np.stack([res.results[b]["out"].reshape(C, 64, 64) for b in range(8)])
    return out.astype(np.float32)


# revision 36
# speedup vs baseline: 1.0019x; 1.0014x over previous
"""Trainium2 Bass kernel for dual-attention block (CAM + SAM + bottleneck).

Contract: kernel(**inputs) takes FULL unsharded inputs
  x     [8, 64, 64, 64] f32
  w_cam [64, 64], w_q [32, 64], w_k [32, 64], w_v [64, 64], w_bn [64, 128]
and returns the full [8, 64, 64, 64] f32 output.

Sharding: data-parallel over batch across 8 NeuronCores (1 image each);
weights replicated. Per-core math (c=64 channels, n=m=4096 spatial):

  CAM: xcT = x.T @ w_cam.T ; Ec = xcT.T @ xcT;
       attn_c = softmax_rows(Ec); bn = ((wbn1 @ attn_c) + I) @ x
       (the +I folds the residual x into the CAM bottleneck matmul)
  SAM: q4/k4 = (w stacked 4x) @ x  -> q,k replicated on 4 partition groups
       S[m,n] = sum_c k[c,m] q[c,n]  (row-tiled K=32 matmuls on disjoint
       32-row quadrants)
       E = exp(S - ln64) in fp8-e4m3  (max|S|=9.05 -> E'max 133 < 240;
           the 1/64 cancels between numerator and denominator)
       acc[c,n] = sum_m W[m,c] E[m,n]  with W = [v'.T | ones] in fp8 and
                  v' = (wbn2 @ w_v) x  (bottleneck conv folded into the
                  value weights on the host), one DoubleRow matmul per
                  m-tile PAIR (K=256) -> rows 0..63 = final (unnormalized)
                  SAM contribution, row 64 = Z
  out = bn + acc[0:64] * (1/Z)
        (1/Z via the custom-DVE fast approx reciprocal -- partition 0
        only, reached through a 2KB Z-row DMA hop -- then broadcast to 64
        partitions by GpSimd partition_broadcast and applied by one DVE
        multiply)

Design (v16, 221us -> ~165.5us).  ScalarE is the bound: 128 exp
activations of [128,1024] at (1024+352)/1.2GHz = 1147ns each = 142.6us;
everything else is scheduled to keep that stream gapless.  The PE runs
cold (1.2 GHz): the HAM clock gate only lifts to 2.4 GHz after ~3.4us of
gapless matmuls and re-throttles on any exp-paced idle, so a warm-up is
pointless -- the schedule instead keeps cold-PE work per exp window
under the exp duration.  Key structure:
  - spool = ONE 3-slot rotation (3 x 2 PSUM banks) for the S tiles: a
    group's bank frees 3 exp-periods ahead, so all 4 K=32 quadrant
    matmuls of a pair issue back-to-back.  bn / m1ps / q-chunk fills
    ride the same rotation (each extra insertion costs ~1us of exp
    stream -- they are batched and placed where gaps already exist).
  - lag-1 DR emission: a pair's DoubleRow matmuls are emitted after the
    NEXT pair's S matmuls, so at block boundaries the next S quad sits
    in the PE queue ahead of the last DRs and the exp stream never
    head-of-line blocks (this alone was worth 11us).
  - block 0 has no PE budget for its DRs next to the wvc (=[v'|cam] 1x1
    conv) production and k-chunk fills, so its E tiles persist in SBUF
    (e0) and its 16 DRs ride block 1's slack; wvc/fill PSUM scratch uses
    the vacc banks, which are idle in block 0.
  - ec (CAM energy) runs in blocks 2-3 (3 matmuls/pair early) and the
    CAM softmax chain at (3,6) so M1T is ready without stalling block 4.
  - vpool rotation: vacc0(A) vacc1(B) | vacc2(A) EC(B) | vacc3(A) |
    CAM, vacc4(B) | vacc5(A) | vacc6(B) | vacc7(A); each claim lands one
    aux-evacuation behind its slot's previous tenant.
  - epilogues are split recip(+2 blocks) / scale(+3..4) / bn+out(+3..4)
    and spread so only block 7's chain remains after the last exp.
  - preamble: wk4T/wq4T first on the sync/scalar DMA queues (DMA
    completion latency is ~3us regardless of size), x in 8 chunks over 3
    queues, only chunk-0 q/k up front with BOTH evacuations on the
    otherwise-idle ScalarE (DVE is congested with casts); first exp
    ~13.5us.
PSUM: spool 3x2 + vacc/EC 2 = 8 banks.
"""

import sys
from contextlib import ExitStack

import numpy as np

if "/opt/trn_rl_repo" not in sys.path:
    sys.path.insert(0, "/opt/trn_rl_repo")

import concourse.bass as bass
import concourse.tile as tile
from concourse import bacc, mybir
from concourse.bass_utils import run_bass_kernel_spmd

F32 = mybir.dt.float32
BF16 = mybir.dt.bfloat16
FP8 = mybir.dt.float8e4

C = 64          # channels
HW = 4096       # 64*64 spatial
NB = 8          # number of 512-wide n blocks
BLK = 512
MT = 32         # m tiles of 128
NG = 16         # groups of 2 m-tiles per n-block
WP = 80         # wt8 per-m-tile stride (65 used; 80 for DoubleRow step%16==0)
NLOG64 = -4.1588830833596715

Exp = mybir.ActivationFunctionType.Exp
DR = mybir.MatmulPerfMode.DoubleRow


def _build_kernel(ctx: ExitStack, tc: tile.TileContext, io: dict):
    nc = tc.nc
    x_d = io["x"]
    out_d = io["out"]

    consts = ctx.enter_context(tc.tile_pool(name="consts", bufs=1))
    bigs = ctx.enter_context(tc.tile_pool(name="bigs", bufs=1))
    epool = ctx.enter_context(tc.tile_pool(name="epool", bufs=4))
    campool = ctx.enter_context(tc.tile_pool(name="campool", bufs=1))
    sampool = ctx.enter_context(tc.tile_pool(name="sampool", bufs=4))
    spool = ctx.enter_context(
        tc.tile_pool(name="spool", bufs=3, space=bass.MemorySpace.PSUM)
    )
    vpool = ctx.enter_context(
        tc.tile_pool(name="vpool", bufs=2, space=bass.MemorySpace.PSUM)
    )

    # ---- weight DMAs first, on the otherwise-idle GpSimd queue (tiny; if
    # they queued behind the 1MB x transfer the first matmul waits ~15us) --
    wq4T = consts.tile([C, 128], BF16)    # (w_q stacked 4x).T
    wk4T = consts.tile([C, 128], BF16)
    wvc = consts.tile([C, 128], BF16)     # [(wbn2 w_v).T | w_cam.T]
    wbn1T = consts.tile([C, C], F32)
    id64 = consts.tile([C, C], BF16)
    zb = consts.tile([128, 1], F32)
    nlog64 = consts.tile([128, 1], F32)   # exp bias: E'=E/64 fits fp8e4 max 240
    dummy = consts.tile([128, 1], F32)

    nc.sync.dma_start(wk4T[:], io["wk4T"][:])
    nc.scalar.dma_start(wq4T[:], io["wq4T"][:])
    nc.gpsimd.dma_start(wvc[:], io["wvc"][:])
    nc.gpsimd.dma_start(wbn1T[:], io["wbn1T"][:])
    nc.gpsimd.dma_start(id64[:], io["id64"][:])

    # ---- x DMA: 8 column chunks round-robin over 3 HWDGE queues (each
    # queue sustains only ~100 GB/s; chunk 0 -- all that the first S
    # matmuls need -- lands first, right behind wk4T/wq4T) ----
    x_sb = bigs.tile([C, HW], F32)
    x_q_of = {0: nc.sync, 2: nc.sync, 6: nc.sync,
              1: nc.scalar, 3: nc.scalar, 7: nc.scalar,
              4: nc.gpsimd, 5: nc.gpsimd}
    for xc_ in (0, 1, 2, 3, 4, 5, 6, 7):
        x_q_of[xc_].dma_start(
            x_sb[:, xc_ * BLK : (xc_ + 1) * BLK],
            x_d[:, xc_ * BLK : (xc_ + 1) * BLK],
        )

    nc.vector.memset(zb[:], 0.0)
    # Trigger the exp ACT-table load right behind the x-DMA issue (overlaps
    # the transfer) instead of in front of the first real exp.
    nc.scalar.activation(dummy[:], zb[:], Exp, bias=zb[:])
    nc.vector.memset(nlog64[:], NLOG64)

    q4 = bigs.tile([128, HW], BF16)
    k4 = bigs.tile([128, HW], BF16)
    wt8 = bigs.tile([128, MT * WP], FP8)   # per m-tile [v'T | ones | pad]
    xct = bigs.tile([128, MT * C], BF16)   # xcT, m-tile-major
    x_bf = bigs.tile([C, HW], BF16)
    e0 = bigs.tile([128, NG * 2 * BLK], FP8)  # block-0 E, consumed in block 1

    # ones column of wt8 (wvc copies below only write cols 0..63)
    nc.vector.memset(
        wt8[:].rearrange("p (t c) -> p t c", c=WP)[:, :, 64:65], 1.0
    )

    # x in bf16 feeds the q4/k4/wvc/bn matmuls at full PE rate.  Only
    # chunks 0-1 are cast up front: casts for chunks that arrive later are
    # emitted inside the loop so they cannot head-of-line-block the DVE
    # queue in front of the k0/q0 evacuations.
    def x_cast(xc_):
        nc.vector.tensor_copy(
            x_bf[:, xc_ * BLK : (xc_ + 1) * BLK], x_sb[:, xc_ * BLK : (xc_ + 1) * BLK]
        )

    x_cast(0)
    x_cast_sched = {(0, 0): [2], (0, 1): [3], (0, 2): [4], (0, 3): [5],
                    (0, 4): [6], (0, 5): [7]}

    # ---- q4 / k4: replicated q,k via stacked-weight 1x1 convs.  Each
    # 2-chunk group is ONE FD=1024 matmul.  Chunks 0-1 of k and q are
    # produced up front; the rest are fill-in groups inside the block
    # loop, each 2+ pairs ahead of its consumption deadline. ----
    def qk_group(which, cch, nch=1, on_scalar=False, pool=None):
        wT, dst = (wk4T, k4) if which == "k" else (wq4T, q4)
        if pool is None:
            ps = spool.tile([128, nch * BLK], F32, tag="s", name="qkps")
        else:
            ps = pool.tile([128, nch * BLK], F32, tag="v", name="qkps")
        for i in range(nch):
            nc.tensor.matmul(
                ps[:, i * BLK : (i + 1) * BLK],
                wT[:],
                x_bf[:, (cch + i) * BLK : (cch + i + 1) * BLK],
                start=True,
                stop=True,
            )
        lo = cch * BLK
        if on_scalar:
            nc.scalar.copy(dst[:, lo : lo + nch * BLK], ps[:])
        else:
            nc.vector.tensor_copy(dst[:, lo : lo + nch * BLK], ps[:])

    qk_group("k", 0, on_scalar=True)
    qk_group("q", 0, on_scalar=True)
    x_cast(1)

    # (block, pair) -> (which, chunk); deadlines: k chunk c is consumed
    # at block-0 pair c; q chunk c at block c.
    qk_fill = {
        (0, 0): [("k", 1), ("k", 2)], (0, 1): [("k", 3)],
        (0, 2): [("k", 4)], (0, 3): [("k", 5)], (0, 4): [("k", 6)],
        (0, 5): [("k", 7)], (0, 6): [("q", 1)],
        (1, 5): [("q", 2)], (2, 2): [("q", 3, 2)],
        (3, 2): [("q", 5, 2)], (4, 2): [("q", 7)],
    }

    state = {}  # EC tile, allocated at block 1 start (vpool slot timing)

    def wvc_group(base, size):
        """xcT and WT (=[v'T|ones]) production for one m-tile group."""
        ps_w = vpool.tile([128, BLK], F32, tag="v", name="wvcps")
        for j in range(size):
            m = base + j
            nc.tensor.matmul(
                ps_w[:, j * 128 : (j + 1) * 128],
                x_bf[:, m * 128 : (m + 1) * 128],
                wvc[:],
                start=True,
                stop=True,
            )
        src = ps_w[:, : size * 128].rearrange("p (j c) -> p j c", c=128)
        wt_dst = wt8[:, base * WP : (base + size) * WP].rearrange(
            "p (j c) -> p j c", c=WP
        )
        with nc.allow_low_precision(reason="v' in fp8 for DoubleRow acc"):
            nc.vector.tensor_copy(wt_dst[:, :, 0:C], src[:, :, 0:C])
        xct_dst = xct[:, base * C : (base + size) * C].rearrange(
            "p (j c) -> p j c", c=C
        )
        with nc.allow_low_precision(reason="xcT in bf16 for cheap ec matmuls"):
            nc.vector.tensor_copy(xct_dst, src[:, :, C : 2 * C])

    def ec_group(base, size):
        EC = state["EC"]
        for j in range(size):
            m = base + j
            nc.tensor.matmul(
                EC[0:C, 0:C],
                xct[:, m * C : (m + 1) * C],
                xct[:, m * C : (m + 1) * C],
                start=(m == 0),
                stop=(m == MT - 1),
            )

    # ---- per-block state for split epilogues ----
    vaccs = [None] * NB
    sam = [None] * NB   # sam65 [65, BLK] f32: rows 0..63 unnorm SAM out, 64 = Z
    rzs = [None] * NB   # rz [1, BLK] bf16 at partition 0
    M1T_sb = campool.tile([C, C], BF16)

    def epilogue_a(nb):
        """At block end: evacuate vacc (recip is emitted separately)."""
        aux = sampool.tile([C + 1, BLK], F32, tag="aux", name="aux")
        nc.vector.tensor_copy(aux[:], vaccs[nb][0 : C + 1, :])
        sam[nb] = aux

    def emit_recip(nb):
        """1/Z for block nb via the fast approx recip + bf16 cast.

        The custom DVE op only works at base partition 0 (and DVE lanes
        cannot move data across partitions), so the Z row is first moved
        from partition 64 to partition 0 by a tiny SBUF->SBUF DMA on the
        otherwise-idle sync queue.
        """
        z0 = sampool.tile([1, BLK], F32, tag="z0", name="z0")
        nc.sync.dma_start(z0[:], sam[nb][C : C + 1, :])
        rz32 = sampool.tile([1, BLK], F32, tag="rz32", name="rz32")
        nc.vector.reciprocal_approx_fast(rz32[:], z0[:])
        rzb = sampool.tile([1, BLK], BF16, tag="rz", name="rzb")
        with nc.allow_low_precision(reason="1/Z in bf16: 0.4% on the SAM term"):
            nc.vector.tensor_copy(rzb[:], rz32[:])
        rzs[nb] = rzb

    def epilogue_b1(nb):
        """Broadcast 1/Z to 64 partitions (GpSimd) and scale the SAM rows."""
        bcast = sampool.tile([C, BLK], BF16, tag="bc", name="bcast")
        nc.gpsimd.partition_broadcast(bcast[:], rzs[nb][:])
        sam_sc = sampool.tile([C, BLK], F32, tag="sc", name="sam_sc")
        nc.vector.tensor_mul(sam_sc[:], sam[nb][0:C, :], bcast[:])
        return sam_sc

    def epilogue_b2(nb, sam_sc):
        """CAM bottleneck (+residual via I) matmul, add SAM term, DMA out."""
        ncol = slice(nb * BLK, (nb + 1) * BLK)
        bn = spool.tile([128, BLK], F32, tag="s", name="bn")
        nc.tensor.matmul(
            bn[0:C, :], M1T_sb[:], x_bf[:, ncol], start=True, stop=True
        )
        o_t = sampool.tile([C, BLK], F32, tag="ot", name="o_t")
        nc.vector.tensor_add(o_t[:], bn[0:C, :], sam_sc[:])
        nc.sync.dma_start(out_d[:, ncol], o_t[:])

    def cam_chain():
        """CAM softmax -> attn_c -> M1T = (wbn1 @ attn_c).T + I"""
        EC = state["EC"]
        negmax = campool.tile([C, 1], F32)
        nc.vector.reduce_max(
            negmax[:], EC[0:C, 0:C], axis=mybir.AxisListType.X, negate=True
        )
        exp_c = campool.tile([C, C], F32)
        nc.scalar.activation(exp_c[:], EC[0:C, 0:C], Exp, bias=negmax[:])
        sum_c = campool.tile([C, 1], F32)
        nc.vector.reduce_sum(sum_c[:], exp_c[:], axis=mybir.AxisListType.X)
        rec_c = campool.tile([C, 1], F32)
        nc.vector.reciprocal(rec_c[:], sum_c[:])
        attn_c = campool.tile([C, C], F32)
        nc.vector.tensor_scalar_mul(attn_c[:], exp_c[:], rec_c[:])
        m1ps = spool.tile([128, BLK], F32, tag="s", name="m1ps")
        nc.tensor.matmul(
            m1ps[0:C, 0:C], attn_c[:], wbn1T[:], start=True, stop=True
        )
        with nc.allow_low_precision(reason="M1T in bf16 feeds a bf16 matmul"):
            nc.vector.tensor_add(M1T_sb[:], m1ps[0:C, 0:C], id64[:])

    # ---- main SAM loop over 8 n-blocks, groups emitted in PAIRS ----
    # Block 0 carries wvc + k-chunk fill-ins instead of its DR matmuls
    # (which would blow its PE budget); its E tiles persist in e0 and the
    # 16 deferred DRs ride block 1's slack.  ec runs in blocks 2-3 and the
    # CAM chain at block 4; the vacc/EC vpool rotation is:
    #   vacc0(A) vacc1(B) | vacc2(A) EC(B) | vacc3(A) | CAM, vacc4(B) |
    #   vacc5(A) | vacc6(B) | vacc7(A)
    # each claim one aux-evacuation behind its slot's previous tenant.
    sc_pend = {}
    # ec m-tile coverage: 3/pair on block-2 pairs 0-3, then 2/pair; all 32
    # done by block-3 pair 5 so the CAM chain can run at (3,6).
    ec_sched = {(2, 0): (0, 3), (2, 1): (3, 3), (2, 2): (6, 3),
                (2, 3): (9, 3), (2, 4): (12, 2), (2, 5): (14, 2),
                (2, 6): (16, 2), (2, 7): (18, 2), (3, 0): (20, 2),
                (3, 1): (22, 2), (3, 2): (24, 2), (3, 3): (26, 2),
                (3, 4): (28, 2), (3, 5): (30, 2)}
    recip_sched = {(2, 0): 0, (2, 4): 1, (3, 0): 2, (4, 0): 3,
                   (5, 0): 4, (6, 0): 5, (7, 0): 6}
    b1_sched = {(4, 4): 0, (5, 1): 1, (5, 5): 2, (6, 1): 3,
                (6, 5): 4, (7, 1): 5, (7, 4): 6}
    b2_sched = {(4, 6): 0, (5, 3): 1, (5, 7): 2, (6, 3): 3,
                (6, 7): 4, (7, 3): 5, (7, 6): 6}
    pend_dr = None  # (nb, [(g, e_t)...], is_last_pair)
    backlog_ramp = [0, 1, 2, 3, 2, 2, 3, 3]
    backlog_next = [0]

    def emit_drs(dnb, ges, _last):
        for g, e_t in ges:
            lhsT = wt8[:, 2 * g * WP : (2 * g + 2) * WP].rearrange(
                "p (two f) -> p two f", two=2
            )[:, :, 0:65]
            rhs = e_t[:].rearrange("p (two f) -> p two f", two=2)
            nc.tensor.matmul(
                vaccs[dnb][0 : C + 1, :],
                lhsT,
                rhs,
                start=(g == 0),
                stop=(g == NG - 1),
                perf_mode=DR,
            )

    for nb in range(NB):
        if nb == 1:
            vaccs[0] = vpool.tile([128, BLK], F32, tag="v", name="vacc0")
        if nb != 0:
            vacc = vpool.tile([128, BLK], F32, tag="v", name="vacc")
            vaccs[nb] = vacc
        if nb == 2:
            # EC right after vacc2: slot B, re-claimed by vacc4 after the
            # CAM chain at block 4 has consumed EC.
            state["EC"] = vpool.tile([128, BLK], F32, tag="v", name="EC")
        ncol = slice(nb * BLK, (nb + 1) * BLK)
        for p in range(NG // 2):
            g0, g1 = 2 * p, 2 * p + 1
            s_ts = []
            for g in (g0, g1):
                s_t = spool.tile([128, 2 * BLK], F32, tag="s", name="s_t")
                s_ts.append(s_t)
                for j in range(2):
                    m = 2 * g + j
                    r = 2 * (g % 2) + j  # row quadrants 0,1 / 2,3
                    nc.tensor.matmul(
                        s_t[:, j * BLK : (j + 1) * BLK],
                        k4[32 * r : 32 * r + 32, m * 128 : (m + 1) * 128],
                        q4[32 * r : 32 * r + 32, ncol],
                        start=True,
                        stop=True,
                        tile_position=(32 * r, 0),
                    )
            if nb == 0:
                wvc_group(2 * g0, 2)
                wvc_group(2 * g1, 2)
            if (nb, p) in ec_sched:
                ec_group(*ec_sched[(nb, p)])
            if (nb, p) == (3, 6):
                cam_chain()
            for xc2 in x_cast_sched.get((nb, p), ()):
                x_cast(xc2)
            for fill in qk_fill.get((nb, p), ()):
                qk_group(*fill, pool=vpool if nb == 0 else None)
            # lag-1 DR emission: the previous pair's DR matmuls are emitted
            # AFTER this pair's S matmuls, so at block boundaries the next
            # block's S quad is in the PE queue ahead of the last DRs and
            # the exp stream never head-of-line blocks on them.
            if pend_dr is not None:
                emit_drs(*pend_dr)
                if pend_dr[2]:
                    epilogue_a(pend_dr[0])
                pend_dr = None
            e_ts = []
            for g, s_t in zip((g0, g1), s_ts):
                if nb == 0:
                    e_t = e0[:, 2 * g * BLK : 2 * (g + 1) * BLK]
                else:
                    e_t = epool.tile([128, 2 * BLK], FP8, tag="e", name="e_t")
                e_ts.append(e_t)
                with nc.allow_low_precision(reason="E in fp8: ~1e-4 on out"):
                    nc.scalar.activation(e_t[:], s_t[:], Exp, bias=nlog64[:])
            if nb != 0:
                pend_dr = (nb, list(zip((g0, g1), e_ts)), p == NG // 2 - 1)
            if nb == 1:
                # block 0's deferred DR matmuls; ramped 0/1/2/3/2/2/3/3 so
                # the early pairs (thin exp-pipeline cushion) stay light
                for _ in range(backlog_ramp[p]):
                    g = backlog_next[0]
                    backlog_next[0] += 1
                    lhsT = wt8[:, 2 * g * WP : (2 * g + 2) * WP].rearrange(
                        "p (two f) -> p two f", two=2
                    )[:, :, 0:65]
                    rhs = e0[:, 2 * g * BLK : 2 * (g + 1) * BLK].rearrange(
                        "p (two f) -> p two f", two=2
                    )
                    nc.tensor.matmul(
                        vaccs[0][0 : C + 1, :],
                        lhsT,
                        rhs,
                        start=(g == 0),
                        stop=(g == NG - 1),
                        perf_mode=DR,
                    )
            if (nb, p) in recip_sched:
                emit_recip(recip_sched[(nb, p)])
            if (nb, p) in b1_sched:
                s = b1_sched[(nb, p)]
                sc_pend[s] = epilogue_b1(s)
            if (nb, p) in b2_sched:
                s = b2_sched[(nb, p)]
                epilogue_b2(s, sc_pend.pop(s))

        if nb == 1:
            epilogue_a(0)
    if pend_dr is not None:
        emit_drs(*pend_dr)
        if pend_dr[2]:
            epilogue_a(pend_dr[0])
        pend_dr = None

    # ---- tail: only block 7's chain remains.  The Z-row DMA reads the
    # PSUM accumulator directly (safe here: vacc7 is never recycled), so
    # the reciprocal chain starts without waiting for the aux copy. ----
    emit_recip(NB - 1)
    sc7 = epilogue_b1(NB - 1)
    epilogue_b2(NB - 1, sc7)


def build_nc():
    nc = bacc.Bacc(
        "TRN2",
        target_bir_lowering=False,
        debug=False,
        enable_asserts=False,
        num_devices=8,
    )
    io = {}
    io["x"] = nc.dram_tensor("x", [C, HW], F32, kind="ExternalInput").ap()
    io["wq4T"] = nc.dram_tensor("wq4T", [C, 128], BF16, kind="ExternalInput").ap()
    io["wk4T"] = nc.dram_tensor("wk4T", [C, 128], BF16, kind="ExternalInput").ap()
    io["wvc"] = nc.dram_tensor("wvc", [C, 128], BF16, kind="ExternalInput").ap()
    io["wbn1T"] = nc.dram_tensor("wbn1T", [C, C], F32, kind="ExternalInput").ap()
    io["id64"] = nc.dram_tensor("id64", [C, C], BF16, kind="ExternalInput").ap()
    io["out"] = nc.dram_tensor("out", [C, HW], F32, kind="ExternalOutput").ap()

    with tile.TileContext(nc) as tc:
        with ExitStack() as ctx:
            _build_kernel(ctx, tc, io)
    nc.compile()
    return nc


def make_in_maps(x, w_cam, w_q, w_k, w_v, w_bn):
    import ml_dtypes

    f = lambda a: np.ascontiguousarray(np.asarray(a, dtype=np.float32))
    fb = lambda a: np.ascontiguousarray(
        np.asarray(a, dtype=np.float32).astype(ml_dtypes.bfloat16)
    )
    w_bn = np.asarray(w_bn, dtype=np.float64)
    w_vp = w_bn[:, C:] @ np.asarray(w_v, dtype=np.float64)  # wbn2 folded into v
    base = {
        "wq4T": fb(np.concatenate([np.asarray(w_q).T] * 4, axis=1)),
        "wk4T": fb(np.concatenate([np.asarray(w_k).T] * 4, axis=1)),
        "wvc": fb(np.concatenate([w_vp.T, np.asarray(w_cam).T], axis=1)),
        "wbn1T": f(w_bn[:, :C].T),
        "id64": fb(np.eye(C)),
    }
    x = np.asarray(x)
    return [dict(base, x=f(x[b].reshape(C, HW))) for b in range(8)]


_NC_CACHE = None


def kernel(x, w_cam, w_q, w_k, w_v, w_bn):
    global _NC_CACHE
    if _NC_CACHE is None:
        _NC_CACHE = build_nc()
    nc = _NC_CACHE
    in_maps = make_in_maps(x, w_cam, w_q, w_k, w_v, w_bn)
    res = run_bass_kernel_spmd(nc, in_maps, list(range(8)))
    out = np.stack([res.results[b]["out"].reshape(C, 64, 64) for b in range(8)])
    return out.astype(np.float32)
